# revision 13
# baseline (speedup 1.0000x reference)
"""GraphSAGE (2-layer, mean aggregation) on 8 Trainium2 NeuronCores.

Strategy (per spec sharding_hint): destination nodes are sharded across the
8 cores; edges are partitioned by destination node. Nodes are sorted by
in-degree and dealt into 49 degree-homogeneous rank blocks; block i
contributes one 128-slot tile to every core and has a uniform per-slot
edge stride D_i (the block's max degree, ~2.4% padding), identical across
cores so one SPMD program serves all 8.

The host performs the neighbor shuffle as pure LAYOUT (no float math):
per-edge source rows are materialized edge-major and slot-grouped
(x_aug[src].T for layer 1; z_aug[src].T from the device-produced bf16 z
table for layer 2, two edges packed per column), so each program streams
its edge data with dense, double-buffered direct DMAs. Per-slot padding
columns point at an appended all-zero row, so a destination tile's
segment sum is exactly a strided free-dim reduction: one vector-engine
tensor_reduce([128, 128 slots, D_i], axis=X) per tile, which also yields
the aggregate directly in [feature, slot] (transposed) form for the dense
stage — no per-edge matmuls, no selection matrices, no indirect DMA
descriptors (the baseline's ~1 us/instruction SWDGE bottleneck), and no
PE transposes. The mean's 1/deg column scale is applied via a rank-1
ones x recip_row outer product on the PE (partition broadcast) followed
by one elementwise multiply per 4-tile group.

Layer 1 also computes, per tile group, z = relu(h) @ W2l.T (bf16) and
s2 = relu(h) @ W2r.T + b2 (f32) so layer 2 is just the 64-wide segment
mean plus the precomputed self term. All float tensor computation
(including every dtype cast) runs on the NeuronCores; the host only does
integer index preprocessing, sharding/layout, and un-sharding.
"""
import sys
from contextlib import ExitStack

import numpy as np

for _p in ("/opt/trn_rl_repo",):
    if _p not in sys.path:
        sys.path.insert(0, _p)

import concourse.bass as bass  # noqa: F401  (kept for parity with prior revisions)
import concourse.tile as tile
from concourse import bacc, mybir
from concourse.bass_utils import run_bass_kernel_spmd


def _ensure_axon_hooks():
    """run_bass_kernel_spmd(trace=True) imports antenv.axon_hooks, which this
    image lacks; install a ctypes-backed hook so tracing works (or degrades
    to a no-op instead of an ImportError)."""
    try:
        import antenv.axon_hooks  # noqa: F401
        return
    except ImportError:
        pass
    import contextlib
    import ctypes
    import types

    def _make_hook():
        try:
            lib = ctypes.CDLL("/opt/axon/libaxon_pjrt.so")
        except OSError:
            return None
        if not hasattr(lib, "axon_start_nrt_profile"):
            return None
        lib.axon_start_nrt_profile.argtypes = [ctypes.POINTER(ctypes.c_int64), ctypes.c_size_t]
        lib.axon_start_nrt_profile.restype = ctypes.c_int64
        lib.axon_stop_nrt_profile.argtypes = [ctypes.c_char_p]
        lib.axon_stop_nrt_profile.restype = ctypes.c_int64

        @contextlib.contextmanager
        def _hook(output_dir, device_ids):
            import jax
            jax.devices()
            if device_ids:
                ids = (ctypes.c_int64 * len(device_ids))(*device_ids)
                rc = lib.axon_start_nrt_profile(ids, len(device_ids))
            else:
                rc = lib.axon_start_nrt_profile(None, 0)
            if rc != 0:
                raise RuntimeError(f"axon_start_nrt_profile rc={rc}")
            try:
                yield
            finally:
                lib.axon_stop_nrt_profile(str(output_dir).encode())

        return _hook

    hook = _make_hook()
    mod = types.ModuleType("antenv.axon_hooks")
    mod.get_axon_ntff_profile_hook = lambda: hook
    mod.set_axon_ntff_profile_hook = lambda h: None
    import antenv
    antenv.axon_hooks = mod
    sys.modules["antenv.axon_hooks"] = mod


_ensure_axon_hooks()


def _run_spmd_retry(nc, in_maps, **kw):
    """One retry for transient NRT device errors (axon cores occasionally
    report EXEC_UNIT_UNRECOVERABLE right after a prior faulted run)."""
    import time
    try:
        return run_bass_kernel_spmd(nc, in_maps, core_ids=list(range(N_CORES)), **kw)
    except Exception:
        time.sleep(15)
        return run_bass_kernel_spmd(nc, in_maps, core_ids=list(range(N_CORES)), **kw)

N_NODES = 50000
N_EDGES = 800000
DIM_IN, DIM_H, DIM_OUT = 128, 256, 64
N_CORES = 8
P = 128
TILES_PER_CORE = 49                      # ceil(50000 / 8 / 128)
BLK = N_CORES * P                        # 1024 nodes per degree-rank block
NPAD_CORE = TILES_PER_CORE * P           # 6272
NPAD_ALL = N_CORES * NPAD_CORE           # 50176
G = 4                                    # tiles per dense/output batch

BF16 = None  # numpy dtype for bfloat16, resolved lazily from mybir

LAST_RESULTS = []   # test harness reads profiling results from here


def _bf16():
    global BF16
    if BF16 is None:
        BF16 = mybir.dt.np(mybir.dt.bfloat16)
    return BF16


def _partition_nodes(deg):
    """Degree-sorted dealing: rank r -> block i = r//1024 (tile index on
    every core), j = r%1024 -> core j%8, slot j//8. Returns per-node core/
    tile/slot plus the uniform per-block stride D (max degree, >=1)."""
    order = np.argsort(-deg, kind="stable")
    core_of = np.empty(N_NODES, np.int64)
    tile_of = np.empty(N_NODES, np.int64)
    slot_of = np.empty(N_NODES, np.int64)
    r = np.arange(N_NODES)
    core_of[order] = (r % BLK) % N_CORES
    tile_of[order] = r // BLK
    slot_of[order] = (r % BLK) // N_CORES
    Ds = np.zeros(TILES_PER_CORE, np.int64)
    for i in range(TILES_PER_CORE):
        blk = deg[order[i * BLK:(i + 1) * BLK]]
        Ds[i] = max(int(blk.max()) if blk.size else 1, 1)
    return core_of, tile_of, slot_of, Ds


def _build_edge_layout(src, dst, core_of, tile_of, slot_of, Ds):
    """Slot-grouped edge-major column layouts.

    Returns per-core: src_flat [CTOT] (layer-1 column -> source node id,
    pad=N_NODES), (se, so) [CTOT2] (layer-2 packed columns: even/odd edge
    source padded-slot ids, pad=NPAD_ALL), deg_row [1, NPAD_CORE].
    Column of (tile i, slot n, edge k): base_i + n*D_i + k  (layer 1) and
    base2_i + n*ceil(D_i/2) + k//2 with k%2 selecting the partition half
    (layer 2).
    """
    D2s = (Ds + 1) // 2
    base = np.concatenate([[0], np.cumsum(P * Ds)[:-1]])
    base2 = np.concatenate([[0], np.cumsum(P * D2s)[:-1]])
    CTOT = int((P * Ds).sum())
    CTOT2 = int((P * D2s).sum())

    esort = np.argsort(dst, kind="stable")
    dsrt, ssrt = dst[esort], src[esort]
    counts = np.bincount(dst, minlength=N_NODES)
    starts = np.concatenate([[0], np.cumsum(counts)[:-1]])
    rank = np.arange(N_EDGES) - np.repeat(starts, counts)

    ec = core_of[dsrt]
    ei = tile_of[dsrt]
    en = slot_of[dsrt]
    col1 = base[ei] + en * Ds[ei] + rank
    col2 = base2[ei] + en * D2s[ei] + rank // 2
    half = rank % 2
    # padded global slot of each source (for the layer-2 z table)
    pos_of = core_of * NPAD_CORE + tile_of * P + slot_of
    spos = pos_of[ssrt]

    src_flats, ses, sos, deg_rows = [], [], [], []
    for c in range(N_CORES):
        m = ec == c
        sf = np.full(CTOT, N_NODES, np.int64)
        sf[col1[m]] = ssrt[m]
        se = np.full(CTOT2, NPAD_ALL, np.int64)
        so_ = np.full(CTOT2, NPAD_ALL, np.int64)
        me, mo = m & (half == 0), m & (half == 1)
        se[col2[me]] = spos[me]
        so_[col2[mo]] = spos[mo]
        src_flats.append(sf)
        ses.append(se)
        sos.append(so_)
        dr = np.zeros((1, NPAD_CORE), np.float32)
        nodes = np.nonzero(core_of == c)[0]
        dr[0, tile_of[nodes] * P + slot_of[nodes]] = counts[nodes]
        deg_rows.append(dr)
    return src_flats, ses, sos, deg_rows, pos_of, CTOT, CTOT2


def _build_layer1(Ds):
    """Layer-1 SPMD program: per tile one segmented vector reduce
    [128, 128, D_i] -> aggT sums; per 4-tile group a rank-1 1/deg scale,
    dense h = relu(W1l@aggT + W1r@selfT + b1), z = W2l@h (bf16),
    s2 = W2r@h + b2 (f32)."""
    Ds = list(Ds)
    CTOT = int(sum(P * d for d in Ds))
    nc = bacc.Bacc("TRN2", target_bir_lowering=False, debug=False,
                   enable_asserts=False, num_devices=N_CORES)
    dt = mybir.dt
    msgs_in = nc.dram_tensor("msgs_in", [P, CTOT], dt.float32, kind="ExternalInput").ap()
    selfT = nc.dram_tensor("selfT", [P, NPAD_CORE], dt.float32, kind="ExternalInput").ap()
    w1lT = nc.dram_tensor("w1lT", [P, DIM_H], dt.float32, kind="ExternalInput").ap()
    w1rT = nc.dram_tensor("w1rT", [P, DIM_H], dt.float32, kind="ExternalInput").ap()
    b1c = nc.dram_tensor("b1c", [P, 2], dt.float32, kind="ExternalInput").ap()
    w2lT = nc.dram_tensor("w2lT", [P, 2 * DIM_OUT], dt.float32, kind="ExternalInput").ap()
    w2rT = nc.dram_tensor("w2rT", [P, 2 * DIM_OUT], dt.float32, kind="ExternalInput").ap()
    b2c = nc.dram_tensor("b2c", [P, 1], dt.float32, kind="ExternalInput").ap()
    deg_row = nc.dram_tensor("deg_row", [1, NPAD_CORE], dt.float32, kind="ExternalInput").ap()
    z_out = nc.dram_tensor("z_out", [DIM_OUT, NPAD_CORE], dt.bfloat16, kind="ExternalOutput").ap()
    s2_out = nc.dram_tensor("s2_out", [DIM_OUT, NPAD_CORE], dt.float32, kind="ExternalOutput").ap()

    with tile.TileContext(nc) as tc:
        with ExitStack() as ctx:
            const = ctx.enter_context(tc.tile_pool(name="const", bufs=1))
            msgp = ctx.enter_context(tc.tile_pool(name="msgp", bufs=2))
            work = ctx.enter_context(tc.tile_pool(name="work", bufs=2))
            gbuf = ctx.enter_context(tc.tile_pool(name="gbuf", bufs=2))
            outp = ctx.enter_context(tc.tile_pool(name="outp", bufs=2))
            psR = ctx.enter_context(tc.tile_pool(name="psR", bufs=2, space="PSUM"))
            psH = ctx.enter_context(tc.tile_pool(name="psH", bufs=1, space="PSUM"))
            psZ = ctx.enter_context(tc.tile_pool(name="psZ", bufs=1, space="PSUM"))

            deg_sb = const.tile([1, NPAD_CORE], dt.float32)
            nc.sync.dma_start(deg_sb[:], deg_row[:, :])
            w1l_f = const.tile([P, DIM_H], dt.float32)
            nc.sync.dma_start(w1l_f[:], w1lT[:, :])
            w1r_f = const.tile([P, DIM_H], dt.float32)
            nc.sync.dma_start(w1r_f[:], w1rT[:, :])
            w2l_f = const.tile([P, 2 * DIM_OUT], dt.float32)
            nc.sync.dma_start(w2l_f[:], w2lT[:, :])
            w2r_f = const.tile([P, 2 * DIM_OUT], dt.float32)
            nc.sync.dma_start(w2r_f[:], w2rT[:, :])
            b1_sb = const.tile([P, 2], dt.float32)
            nc.sync.dma_start(b1_sb[:], b1c[:, :])
            b2_sb = const.tile([P, 1], dt.float32)
            nc.sync.dma_start(b2_sb[:], b2c[:, :])
            self_f = const.tile([P, NPAD_CORE], dt.float32)
            nc.sync.dma_start(self_f[:], selfT[:, :])

            w1l_sb = const.tile([P, DIM_H], dt.bfloat16)
            nc.vector.tensor_copy(w1l_sb[:], w1l_f[:])
            w1r_sb = const.tile([P, DIM_H], dt.bfloat16)
            nc.vector.tensor_copy(w1r_sb[:], w1r_f[:])
            w2l_sb = const.tile([P, 2 * DIM_OUT], dt.bfloat16)
            nc.vector.tensor_copy(w2l_sb[:], w2l_f[:])
            w2r_sb = const.tile([P, 2 * DIM_OUT], dt.bfloat16)
            nc.vector.tensor_copy(w2r_sb[:], w2r_f[:])
            self_sb = const.tile([P, NPAD_CORE], dt.bfloat16)
            nc.vector.tensor_copy(self_sb[:], self_f[:])

            ones1 = const.tile([1, P], dt.bfloat16)
            nc.vector.memset(ones1[:], 1.0)
            rec_f = const.tile([1, NPAD_CORE], dt.float32)
            nc.vector.tensor_scalar_max(rec_f[:], deg_sb[:], 1.0)
            nc.vector.reciprocal(rec_f[:], rec_f[:])
            rec_bf = const.tile([1, NPAD_CORE], dt.bfloat16)
            nc.vector.tensor_copy(rec_bf[:], rec_f[:])

            cbase = 0
            for g0 in range(0, TILES_PER_CORE, G):
                n_t = min(G, TILES_PER_CORE - g0)
                W = n_t * P
                aggs_cat = gbuf.tile([P, G * P], dt.float32)
                for ti in range(n_t):
                    t = g0 + ti
                    D = Ds[t]
                    m3 = msgp.tile([P, P, D], dt.float32)
                    nc.sync.dma_start(m3[:, :, :], msgs_in[:, cbase:cbase + P * D])
                    cbase += P * D
                    nc.vector.tensor_reduce(
                        out=aggs_cat[:, ti * P:(ti + 1) * P], in_=m3[:, :, :],
                        axis=mybir.AxisListType.X, op=mybir.AluOpType.add)
                # mean scale: recf[p, col] = 1 * rec_row[col]; aggT = sums * recf
                rec_ps = psR.tile([P, G * P], dt.float32)
                nc.tensor.matmul(out=rec_ps[:, :W], lhsT=ones1[:],
                                 rhs=rec_bf[:, g0 * P:g0 * P + W], start=True, stop=True)
                aggT_cat = gbuf.tile([P, G * P], dt.bfloat16, name="aggT")
                nc.vector.tensor_tensor(out=aggT_cat[:, :W], in0=aggs_cat[:, :W],
                                        in1=rec_ps[:, :W], op=mybir.AluOpType.mult)
                # dense stage, batched over the group (moving dim W<=512)
                hT = []
                for so in range(2):
                    h_ps = psH.tile([P, G * P], dt.float32)
                    nc.tensor.matmul(out=h_ps[:, :W], lhsT=w1l_sb[:, so * P:(so + 1) * P],
                                     rhs=aggT_cat[:, :W], start=True, stop=False)
                    nc.tensor.matmul(out=h_ps[:, :W], lhsT=w1r_sb[:, so * P:(so + 1) * P],
                                     rhs=self_sb[:, g0 * P:g0 * P + W], start=False, stop=True)
                    h_sb = work.tile([P, G * P], dt.bfloat16, name=f"h{so}")
                    nc.scalar.activation(h_sb[:, :W], h_ps[:, :W],
                                         mybir.ActivationFunctionType.Relu,
                                         bias=b1_sb[:, so:so + 1], scale=1.0)
                    hT.append(h_sb)
                z_ps = psZ.tile([DIM_OUT, G * P], dt.float32)
                for si in range(2):
                    nc.tensor.matmul(out=z_ps[:, :W], lhsT=w2l_sb[:, si * DIM_OUT:(si + 1) * DIM_OUT],
                                     rhs=hT[si][:, :W], start=(si == 0), stop=(si == 1))
                z_sb = outp.tile([DIM_OUT, G * P], dt.bfloat16)
                nc.vector.tensor_copy(z_sb[:, :W], z_ps[:, :W])
                nc.sync.dma_start(z_out[:, g0 * P:g0 * P + W], z_sb[:, :W])
                s_ps = psZ.tile([DIM_OUT, G * P], dt.float32)
                for si in range(2):
                    nc.tensor.matmul(out=s_ps[:, :W], lhsT=w2r_sb[:, si * DIM_OUT:(si + 1) * DIM_OUT],
                                     rhs=hT[si][:, :W], start=(si == 0), stop=(si == 1))
                s_sb = outp.tile([DIM_OUT, G * P], dt.float32, name="ssb")
                nc.scalar.activation(s_sb[:, :W], s_ps[:, :W],
                                     mybir.ActivationFunctionType.Identity,
                                     bias=b2_sb[:DIM_OUT, 0:1], scale=1.0)
                nc.sync.dma_start(s2_out[:, g0 * P:g0 * P + W], s_sb[:, :W])
    nc.compile()
    return nc


def _build_layer2(Ds):
    """Layer-2 SPMD program: per tile one segmented reduce over packed
    bf16 z messages (two edges per column, halves folded by one add),
    rank-1 1/deg scale, plus precomputed self term."""
    Ds = list(Ds)
    D2s = [(d + 1) // 2 for d in Ds]
    CTOT2 = int(sum(P * d for d in D2s))
    nc = bacc.Bacc("TRN2", target_bir_lowering=False, debug=False,
                   enable_asserts=False, num_devices=N_CORES)
    dt = mybir.dt
    msgs_in = nc.dram_tensor("msgs_in", [P, CTOT2], dt.bfloat16, kind="ExternalInput").ap()
    s2T = nc.dram_tensor("s2T", [DIM_OUT, NPAD_CORE], dt.float32, kind="ExternalInput").ap()
    deg_row = nc.dram_tensor("deg_row", [1, NPAD_CORE], dt.float32, kind="ExternalInput").ap()
    out2 = nc.dram_tensor("out2", [DIM_OUT, NPAD_CORE], dt.float32, kind="ExternalOutput").ap()

    with tile.TileContext(nc) as tc:
        with ExitStack() as ctx:
            const = ctx.enter_context(tc.tile_pool(name="const", bufs=1))
            msgp = ctx.enter_context(tc.tile_pool(name="msgp", bufs=2))
            work = ctx.enter_context(tc.tile_pool(name="work", bufs=2))
            gbuf = ctx.enter_context(tc.tile_pool(name="gbuf", bufs=2))
            psR = ctx.enter_context(tc.tile_pool(name="psR", bufs=2, space="PSUM"))
            psF = ctx.enter_context(tc.tile_pool(name="psF", bufs=2, space="PSUM"))

            deg_sb = const.tile([1, NPAD_CORE], dt.float32)
            nc.sync.dma_start(deg_sb[:], deg_row[:, :])
            s2_sb = const.tile([DIM_OUT, NPAD_CORE], dt.float32)
            nc.sync.dma_start(s2_sb[:], s2T[:, :])

            ones1 = const.tile([1, DIM_OUT], dt.bfloat16)
            nc.vector.memset(ones1[:], 1.0)
            rec_f = const.tile([1, NPAD_CORE], dt.float32)
            nc.vector.tensor_scalar_max(rec_f[:], deg_sb[:], 1.0)
            nc.vector.reciprocal(rec_f[:], rec_f[:])
            rec_bf = const.tile([1, NPAD_CORE], dt.bfloat16)
            nc.vector.tensor_copy(rec_bf[:], rec_f[:])
            # fold matrix: [I64; I64] stacked -> PE folds the packed halves
            fold = const.tile([P, DIM_OUT], dt.bfloat16)
            nc.gpsimd.memset(fold[:], 0.0)
            nc.gpsimd.affine_select(
                out=fold[:DIM_OUT, :], in_=fold[:DIM_OUT, :],
                compare_op=mybir.AluOpType.not_equal, fill=1.0,
                base=0, pattern=[[-1, DIM_OUT]], channel_multiplier=1)
            nc.gpsimd.affine_select(
                out=fold[DIM_OUT:, :], in_=fold[DIM_OUT:, :],
                compare_op=mybir.AluOpType.not_equal, fill=1.0,
                base=0, pattern=[[-1, DIM_OUT]], channel_multiplier=1)

            cbase = 0
            for g0 in range(0, TILES_PER_CORE, G):
                n_t = min(G, TILES_PER_CORE - g0)
                W = n_t * P
                rec_ps = psR.tile([DIM_OUT, G * P], dt.float32)
                nc.tensor.matmul(out=rec_ps[:, :W], lhsT=ones1[:],
                                 rhs=rec_bf[:, g0 * P:g0 * P + W], start=True, stop=True)
                rec_sb = work.tile([DIM_OUT, G * P], dt.float32, name="recsb")
                nc.vector.tensor_copy(rec_sb[:, :W], rec_ps[:, :W])
                o_cat = gbuf.tile([DIM_OUT, G * P], dt.float32, name="ocat")
                for ti in range(n_t):
                    t = g0 + ti
                    D2 = D2s[t]
                    m3 = msgp.tile([P, P, D2], dt.bfloat16)
                    nc.sync.dma_start(m3[:, :, :], msgs_in[:, cbase:cbase + P * D2])
                    cbase += P * D2
                    pack = work.tile([P, P], dt.float32)
                    nc.vector.tensor_reduce(
                        out=pack[:], in_=m3[:, :, :],
                        axis=mybir.AxisListType.X, op=mybir.AluOpType.add)
                    pack_bf = work.tile([P, P], dt.bfloat16, name="packbf")
                    nc.vector.tensor_copy(pack_bf[:], pack[:])
                    # fold the even/odd halves across partitions on the PE
                    f_ps = psF.tile([DIM_OUT, P], dt.float32)
                    nc.tensor.matmul(out=f_ps[:], lhsT=fold[:], rhs=pack_bf[:],
                                     start=True, stop=True)
                    nc.vector.tensor_tensor(out=o_cat[:, ti * P:(ti + 1) * P],
                                            in0=f_ps[:],
                                            in1=rec_sb[:, ti * P:(ti + 1) * P],
                                            op=mybir.AluOpType.mult)
                nc.vector.tensor_add(o_cat[:, :W], o_cat[:, :W],
                                     s2_sb[:, g0 * P:g0 * P + W])
                nc.sync.dma_start(out2[:, g0 * P:g0 * P + W], o_cat[:, :W])
    nc.compile()
    return nc


_PROG_CACHE = {}


def _get_programs(Ds):
    key = tuple(Ds)
    if key not in _PROG_CACHE:
        _PROG_CACHE[key] = (_build_layer1(Ds), _build_layer2(Ds))
    return _PROG_CACHE[key]


def _pack_w(w):
    """[f_out, f_in] weight -> [128, SI*f_out] with [p, si*f_out+f] = w[f, si*128+p]."""
    f_out, f_in = w.shape
    si = f_in // P
    return np.ascontiguousarray(np.hstack([w.T[i * P:(i + 1) * P, :] for i in range(si)]), dtype=np.float32)


def _pack_b(b, cols):
    out = np.zeros((P, cols), np.float32)
    for i in range(cols):
        seg = b[i * P:(i + 1) * P]
        out[:seg.shape[0], i] = seg
    return out


def kernel(x, edge_index, W1l, W1r, b1, W2l, W2r, b2):
    global LAST_RESULTS
    LAST_RESULTS = []
    bf16 = _bf16()
    x = np.asarray(x, np.float32)
    src = np.asarray(edge_index[0], np.int64)
    dst = np.asarray(edge_index[1], np.int64)

    deg = np.bincount(dst, minlength=N_NODES)
    core_of, tile_of, slot_of, Ds = _partition_nodes(deg)
    src_flats, ses, sos, deg_rows, pos_of, CTOT, CTOT2 = _build_edge_layout(
        src, dst, core_of, tile_of, slot_of, Ds)

    l1, l2 = _get_programs(Ds)

    trace = bool(int(__import__("os").environ.get("BASS_TRACE", "0") or 0))
    tkw = dict(trace=True, tmpdir=None) if trace else {}

    x_aug = np.vstack([x, np.zeros((1, DIM_IN), np.float32)])

    # per-core self features, transposed (pure layout)
    selfTs = []
    for c in range(N_CORES):
        sT = np.zeros((NPAD_CORE, DIM_IN), np.float32)
        nodes = np.nonzero(core_of == c)[0]
        sT[tile_of[nodes] * P + slot_of[nodes]] = x[nodes]
        selfTs.append(np.ascontiguousarray(sT.T))

    w1l_p, w1r_p = _pack_w(np.asarray(W1l)), _pack_w(np.asarray(W1r))
    w2l_p, w2r_p = _pack_w(np.asarray(W2l)), _pack_w(np.asarray(W2r))
    b1_p = _pack_b(np.asarray(b1), 2)
    b2_p = _pack_b(np.asarray(b2), 1)

    in_maps = []
    for c in range(N_CORES):
        m = np.ascontiguousarray(x_aug[src_flats[c]].T)   # [128, CTOT] f32
        in_maps.append({
            "msgs_in": m,
            "selfT": selfTs[c],
            "w1lT": w1l_p, "w1rT": w1r_p, "b1c": b1_p,
            "w2lT": w2l_p, "w2rT": w2r_p, "b2c": b2_p,
            "deg_row": deg_rows[c],
        })
    r1 = _run_spmd_retry(l1, in_maps, **tkw)
    LAST_RESULTS.append(r1)

    # device-produced bf16 z table (+ zero row for padding), node-major
    znode = np.concatenate([np.ascontiguousarray(np.asarray(r1.results[c]["z_out"]).T)
                            for c in range(N_CORES)], axis=0)  # [50176, 64] bf16
    z_aug = np.vstack([znode, np.zeros((1, DIM_OUT), bf16)])

    in_maps2 = []
    for c in range(N_CORES):
        m2 = np.concatenate([np.ascontiguousarray(z_aug[ses[c]].T),
                             np.ascontiguousarray(z_aug[sos[c]].T)], axis=0)  # [128, CTOT2]
        in_maps2.append({
            "msgs_in": np.ascontiguousarray(m2),
            "s2T": np.asarray(r1.results[c]["s2_out"]),
            "deg_row": deg_rows[c],
        })
    r2 = _run_spmd_retry(l2, in_maps2, **tkw)
    LAST_RESULTS.append(r2)

    big = np.concatenate([np.asarray(r2.results[c]["out2"]) for c in range(N_CORES)], axis=1)
    out = np.ascontiguousarray(big[:, pos_of[np.arange(N_NODES)]].T, dtype=np.float32)
    return out
